# revision 3
# baseline (speedup 1.0000x reference)
"""Differential cross-attention kernel for Trainium2, 8-core data-parallel.

Per core (one batch element b):
  qT = (Wq.T/16) @ geneT            [E, NG-chunk]   (fp32r matmuls)
  kT = Wk.T @ subT                  [E, NS]
  v  = sub @ Wv.T                   [NS, E]
  S_i = q_i k_i^T                   [128m, NS] in PSUM  (i = head 1, 2)
  P_i = exp(S_i), d_i = rowsum      (ACT, accum_out)
  diff = P1/d1 - lam*P2/d2          (DVE)  -> HBM output + PE-transpose
  O = diff @ v                      (via transposed diff blocks)
  out = RMSNorm(O) @ (w*(1-l0)*Wo.T)

Inputs are staged on host: gene/substructure transposed per batch, weights
pre-transposed & pre-scaled, lambda computed on host (tiny dot products).
"""
import math

import numpy as np

import concourse.bass as bass
import concourse.mybir as mybir
import concourse.tile as tile
from concourse import bacc
from concourse import bass_utils
from concourse.masks import make_identity

N_CORES = 8
B, NG, NS, E = 8, 4096, 1024, 512
H = E // 2                     # 256, per-head dim
LAMBDA_INIT = 0.8 - 0.6 * math.exp(-0.3 * 0.0)   # depth 0 -> 0.2
RMS_EPS = 1e-5
P = 128                        # partitions
KI = E // P                    # 4 e_in tiles
EO = E // P                    # 4 e_out tiles
NB = NS // P                   # 8 kv tiles
CHUNK = 512                    # m tokens per chunk
NCH = NG // CHUNK              # 8 chunks
JT = CHUNK // P                # 4 m-subtiles per chunk

F32 = mybir.dt.float32
F32R = mybir.dt.float32r
AF = mybir.ActivationFunctionType
ALU = mybir.AluOpType


def build_kernel():
    nc = bacc.Bacc("TRN2", target_bir_lowering=False, debug=False,
                   num_devices=N_CORES)
    geneT = nc.dram_tensor("geneT", [E, NG], F32R, kind="ExternalInput").ap()
    subT = nc.dram_tensor("subT", [E, NS], F32R, kind="ExternalInput").ap()
    wqT = nc.dram_tensor("wqT", [E, E], F32R, kind="ExternalInput").ap()
    wkT = nc.dram_tensor("wkT", [E, E], F32R, kind="ExternalInput").ap()
    wvT = nc.dram_tensor("wvT", [E, E], F32R, kind="ExternalInput").ap()
    woT = nc.dram_tensor("woT", [E, E], F32R, kind="ExternalInput").ap()
    lam = nc.dram_tensor("lam", [P, 1], F32, kind="ExternalInput").ap()
    out_d = nc.dram_tensor("out", [NG, E], F32, kind="ExternalOutput").ap()
    diff_d = nc.dram_tensor("diff", [NG, NS], F32, kind="ExternalOutput").ap()

    with tile.TileContext(nc) as tc:
        emit(tc, geneT, subT, wqT, wkT, wvT, woT, lam, out_d, diff_d)
    nc.compile()
    return nc


def emit(tc, geneT, subT, wqT, wkT, wvT, woT, lam, out_d, diff_d):
    nc = tc.nc
    from contextlib import ExitStack
    with ExitStack() as ctx:
        consts = ctx.enter_context(tc.tile_pool(name="consts", bufs=1))
        kvp = ctx.enter_context(tc.tile_pool(name="kvp", bufs=1))
        gpool = ctx.enter_context(tc.tile_pool(name="gpool", bufs=2))
        qpool = ctx.enter_context(tc.tile_pool(name="qpool", bufs=2))
        ppool = ctx.enter_context(tc.tile_pool(name="ppool", bufs=2))
        dfpool = ctx.enter_context(tc.tile_pool(name="dfpool", bufs=3))
        dTpool = ctx.enter_context(tc.tile_pool(name="dTpool", bufs=2))
        sqpool = ctx.enter_context(tc.tile_pool(name="sqpool", bufs=2))
        nopool = ctx.enter_context(tc.tile_pool(name="nopool", bufs=2))
        ypool = ctx.enter_context(tc.tile_pool(name="ypool", bufs=2))
        dpool = ctx.enter_context(tc.tile_pool(name="dpool", bufs=24))
        ps_s = ctx.enter_context(tc.tile_pool(name="ps_s", bufs=2, space="PSUM"))
        ps_tt = ctx.enter_context(tc.tile_pool(name="ps_tt", bufs=1, space="PSUM"))
        ps_sm = ctx.enter_context(tc.tile_pool(name="ps_sm", bufs=2, space="PSUM"))

        # ---- constants -------------------------------------------------
        w_sb = {}
        for name, src in (("wq", wqT), ("wk", wkT), ("wv", wvT), ("wo", woT)):
            t = consts.tile([P, KI, E], F32R, tag=name)
            nc.sync.dma_start(out=t[:], in_=src.rearrange("(ki p) e -> p ki e", p=P))
            w_sb[name] = t
        ident = consts.tile([P, P], F32, tag="ident")
        make_identity(nc, ident[:])
        lam_sb = consts.tile([P, 1], F32, tag="lam")
        nc.sync.dma_start(out=lam_sb[:], in_=lam)
        eps_sb = consts.tile([P, 1], F32, tag="eps")
        nc.vector.memset(eps_sb[:], RMS_EPS)

        # ---- kv setup: kT [E, NS] and v [NS, E] ------------------------
        subT_sb = kvp.tile([P, KI, NS], F32R, tag="subT")
        nc.sync.dma_start(out=subT_sb[:],
                          in_=subT.rearrange("(ki p) n -> p ki n", p=P))
        kT_sb = kvp.tile([P, EO, NS], F32R, tag="kT")
        v_sb = kvp.tile([P, NB, E], F32R, tag="v")

        for eo in range(EO):
            for nch in range(NS // 512):
                pk = ps_sm.tile([P, 512], F32, tag="sm")
                for ki in range(KI):
                    nc.tensor.matmul(
                        pk[:],
                        w_sb["wk"][:, ki, eo * P:(eo + 1) * P],
                        subT_sb[:, ki, nch * 512:(nch + 1) * 512],
                        start=(ki == 0), stop=(ki == KI - 1))
                nc.scalar.copy(kT_sb[:, eo, nch * 512:(nch + 1) * 512], pk[:])

        for nb in range(NB):
            pv = ps_sm.tile([P, 512], F32, tag="sm")
            for ki in range(KI):
                nc.tensor.matmul(
                    pv[:],
                    subT_sb[:, ki, nb * P:(nb + 1) * P],
                    w_sb["wv"][:, ki, :],
                    start=(ki == 0), stop=(ki == KI - 1))
            nc.scalar.copy(v_sb[:, nb, :], pv[:])

        # ---- main loop over m-chunks ----------------------------------
        for c in range(NCH):
            gT = gpool.tile([P, KI, CHUNK], F32R, tag="gT")
            nc.sync.dma_start(
                out=gT[:],
                in_=geneT[:, c * CHUNK:(c + 1) * CHUNK].rearrange(
                    "(ki p) m -> p ki m", p=P))
            qT = qpool.tile([P, EO, CHUNK], F32R, tag="qT")
            for eo in range(EO):
                pq = ps_sm.tile([P, 512], F32, tag="sm")
                for ki in range(KI):
                    nc.tensor.matmul(
                        pq[:],
                        w_sb["wq"][:, ki, eo * P:(eo + 1) * P],
                        gT[:, ki, :],
                        start=(ki == 0), stop=(ki == KI - 1))
                nc.scalar.copy(qT[:, eo, :], pq[:])

            for j in range(JT):
                m0 = c * CHUNK + j * P
                # S1/S2 logits [128, NS] in PSUM
                s1 = ps_s.tile([P, NS], F32, tag="s")
                s2 = ps_s.tile([P, NS], F32, tag="s")
                for hk in range(2):
                    for nch in range(NS // 512):
                        nc.tensor.matmul(
                            s1[:, nch * 512:(nch + 1) * 512],
                            qT[:, hk, j * P:(j + 1) * P],
                            kT_sb[:, hk, nch * 512:(nch + 1) * 512],
                            start=(hk == 0), stop=(hk == 1))
                for hk in range(2):
                    for nch in range(NS // 512):
                        nc.tensor.matmul(
                            s2[:, nch * 512:(nch + 1) * 512],
                            qT[:, 2 + hk, j * P:(j + 1) * P],
                            kT_sb[:, 2 + hk, nch * 512:(nch + 1) * 512],
                            start=(hk == 0), stop=(hk == 1))

                # softmax numerators + row sums
                p1 = ppool.tile([P, NS], F32, tag="p1")
                d1 = dpool.tile([P, 1], F32, tag="d")
                nc.scalar.activation(out=p1[:], in_=s1[:], func=AF.Exp,
                                     accum_out=d1[:])
                p2 = ppool.tile([P, NS], F32, tag="p2")
                d2 = dpool.tile([P, 1], F32, tag="d")
                nc.scalar.activation(out=p2[:], in_=s2[:], func=AF.Exp,
                                     accum_out=d2[:])
                r1 = dpool.tile([P, 1], F32, tag="d")
                nc.vector.reciprocal(r1[:], d1[:])
                r2 = dpool.tile([P, 1], F32, tag="d")
                nc.vector.reciprocal(r2[:], d2[:])
                r2l = dpool.tile([P, 1], F32, tag="d")
                nc.vector.tensor_mul(r2l[:], r2[:], lam_sb[:])
                # p2 <- p2 * (lam/d2) in place; diff = p1*r1 - p2
                nc.vector.tensor_scalar_mul(p2[:], p2[:], r2l[:])
                diff = dfpool.tile([P, NS], F32, tag="diff")
                nc.vector.scalar_tensor_tensor(
                    out=diff[:], in0=p1[:], scalar=r1[:], in1=p2[:],
                    op0=ALU.mult, op1=ALU.subtract)
                nc.sync.dma_start(out=diff_d[m0:m0 + P, :], in_=diff[:])

                # transpose diff -> diffT (PE), copy to SBUF
                tt = ps_tt.tile([P, NS], F32, tag="tt")
                for nb in range(NB):
                    nc.tensor.transpose(tt[:, nb * P:(nb + 1) * P],
                                        diff[:, nb * P:(nb + 1) * P], ident[:])
                dT = dTpool.tile([P, NS], F32R, tag="dT")
                nc.scalar.copy(dT[:], tt[:])

                # O = diff @ v  [128, E]
                po = ps_sm.tile([P, E], F32, tag="sm")
                for nb in range(NB):
                    nc.tensor.matmul(
                        po[:],
                        dT[:, nb * P:(nb + 1) * P],
                        v_sb[:, nb, :],
                        start=(nb == 0), stop=(nb == NB - 1))

                # RMS norm: rstd = 1/sqrt(mean(O^2)+eps)
                sq = sqpool.tile([P, E], F32, tag="sq")
                ssq = dpool.tile([P, 1], F32, tag="d")
                nc.scalar.activation(out=sq[:], in_=po[:], func=AF.Square,
                                     accum_out=ssq[:])
                st = dpool.tile([P, 1], F32, tag="d")
                nc.scalar.activation(out=st[:], in_=ssq[:], func=AF.Sqrt,
                                     scale=1.0 / E, bias=eps_sb[:])
                rstd = dpool.tile([P, 1], F32, tag="d")
                nc.vector.reciprocal(rstd[:], st[:])
                no = nopool.tile([P, E], F32, tag="no")
                nc.vector.tensor_scalar_mul(no[:], po[:], rstd[:])

                # transpose normed O, project with Wo
                nt = ps_sm.tile([P, E], F32, tag="sm")
                for eb in range(EO):
                    nc.tensor.transpose(nt[:, eb * P:(eb + 1) * P],
                                        no[:, eb * P:(eb + 1) * P], ident[:])
                noT = nopool.tile([P, E], F32R, tag="noT")
                nc.vector.tensor_copy(noT[:], nt[:])
                py = ps_sm.tile([P, E], F32, tag="sm")
                for eb in range(EO):
                    nc.tensor.matmul(
                        py[:],
                        noT[:, eb * P:(eb + 1) * P],
                        w_sb["wo"][:, eb, :],
                        start=(eb == 0), stop=(eb == EO - 1))
                y = ypool.tile([P, E], F32, tag="y")
                nc.scalar.copy(y[:], py[:])
                nc.sync.dma_start(out=out_d[m0:m0 + P, :], in_=y[:])


# ---------------------------------------------------------------------------
_NC = None


def get_nc():
    global _NC
    if _NC is None:
        _NC = build_kernel()
    return _NC


def stage_inputs(gene, substructure, Wq, Wk, Wv, Wo,
                 lambda_q1, lambda_k1, lambda_q2, lambda_k2, rms_weight):
    gene = np.asarray(gene, np.float32)
    substructure = np.asarray(substructure, np.float32)
    scaling = H ** -0.5
    lam_full = (math.exp(float(np.sum(np.asarray(lambda_q1, np.float64) *
                                      np.asarray(lambda_k1, np.float64))))
                - math.exp(float(np.sum(np.asarray(lambda_q2, np.float64) *
                                        np.asarray(lambda_k2, np.float64))))
                + LAMBDA_INIT)
    wqT = np.ascontiguousarray(np.asarray(Wq, np.float32).T * scaling)
    wkT = np.ascontiguousarray(np.asarray(Wk, np.float32).T)
    wvT = np.ascontiguousarray(np.asarray(Wv, np.float32).T)
    woT = np.ascontiguousarray(
        np.asarray(rms_weight, np.float32)[:, None]
        * np.asarray(Wo, np.float32).T * (1.0 - LAMBDA_INIT))
    lam_tile = np.full((P, 1), lam_full, np.float32)
    in_maps = []
    for b in range(N_CORES):
        in_maps.append({
            "geneT": np.ascontiguousarray(gene[b].T),
            "subT": np.ascontiguousarray(substructure[b].T),
            "wqT": wqT, "wkT": wkT, "wvT": wvT, "woT": woT,
            "lam": lam_tile,
        })
    return in_maps


def run(in_maps, trace=False, **kw):
    nc = get_nc()
    last_err = None
    for attempt in range(3):
        try:
            return bass_utils.run_bass_kernel_spmd(
                nc, in_maps, core_ids=list(range(N_CORES)), trace=trace, **kw)
        except Exception as e:  # transient device errors on first touch
            last_err = e
    raise last_err


def kernel(**inputs):
    in_maps = stage_inputs(**inputs)
    res = run(in_maps, trace=False)
    out = np.stack([res.results[b]["out"] for b in range(N_CORES)])
    diff = np.stack([res.results[b]["diff"] for b in range(N_CORES)])
    return out, diff
